# revision 31
# baseline (speedup 1.0000x reference)
"""TransformerXL relative attention on 8 TRN2 NeuronCores (batch-parallel).

v13: single fused pipeline, rel_shift entirely on-chip, HAM-aware schedule.

Per-core (one batch element):
  - warmup matmul burst releases the HAM clock gate before real work
  - projections as 8-matmul PSUM waves: rT, q(+biases), then kT (i-outer,
    interleaved with the pair-0 attention chain), then v (interleaved with
    the pair-1 chain); weights/activations stream in bf16 via gpsimd DMA
  - position logits P per (pair, head, q-tile) -> [128,512] PSUM (never-
    read leading columns skipped) -> bf16 pst ring slots (3 separate slot
    tensors so the skewed read's conservative span stays slot-local);
    rows padded to 1536 with -30000
  - rel_shift via skewed SBUF->SBUF DMA: flat AP with partition stride
    (row_pitch - 1) reads P[q, j - q + 511]; masked region lands on the
    pad -> exp -> 0 (mask for free)
  - identity-FIRST scores: shifted P injected into [128,512] sc psums one
    step ahead (6-deep ring); content matmuls (head-pair quadrant-packed)
    close the accumulation group so each step's Exp depends only on the
    first matmuls of that step -- keeps the ACT stream dense and the PE
    out of HAM re-throttle
  - Exp on ACT (nothing else on ACT's loop lane); denominators via DVE
    tensor_reduce; batched reciprocals; normalize on DVE; attn transposed
    via DMA-xbar on the sync queue
  - PV in 4-matmul chunks spread across the next pair's steps (dedicated
    psum ring), outT drained as copy + add; output projection tail
"""

import sys

if "/opt/trn_rl_repo" not in sys.path:
    sys.path.insert(0, "/opt/trn_rl_repo")

import numpy as np

B, Q, MEM, D, H, S = 8, 512, 512, 1024, 16, 64
R = Q + MEM  # 1024
PIT = 1536  # padded row pitch of a pst slot (1024 data + 512 pad)
PAD_VAL = -30000.0
NKD = D // 128  # 8 contraction tiles
NI = D // 128  # 8 hs-tiles
NQT = Q // 128  # 4 q-tiles
NRT = R // 128  # 8 r-tiles
NP = H // 2  # 8 head pairs
NSLOT = 3  # pst ring slots

_CACHE = {}


def _build_nc():
    import concourse.bass as bass_mod
    import concourse.mybir as mybir
    import concourse.tile as tile
    from concourse import bacc
    from concourse.bass import ds
    from concourse.masks import make_identity

    f32 = mybir.dt.float32
    bf16 = mybir.dt.bfloat16
    AF = mybir.ActivationFunctionType

    nc = bacc.Bacc("TRN2", target_bir_lowering=False)

    qTin = nc.dram_tensor("qT", [D, Q], bf16, kind="ExternalInput")
    refTin = nc.dram_tensor("refT", [D, R], bf16, kind="ExternalInput")
    posTin = nc.dram_tensor("posT", [D, R], bf16, kind="ExternalInput")
    Wq_d = nc.dram_tensor("Wq", [D, D], bf16, kind="ExternalInput")
    Wk_d = nc.dram_tensor("Wk", [D, D], bf16, kind="ExternalInput")
    Wv_d = nc.dram_tensor("Wv", [D, D], bf16, kind="ExternalInput")
    Wr_d = nc.dram_tensor("Wr", [D, D], bf16, kind="ExternalInput")
    Wo_d = nc.dram_tensor("Wo", [D, D], bf16, kind="ExternalInput")
    cb_d = nc.dram_tensor("cb", [128, NI], f32, kind="ExternalInput")
    pb_d = nc.dram_tensor("pb", [128, NI], f32, kind="ExternalInput")
    out_d = nc.dram_tensor("out", [Q, D], f32, kind="ExternalOutput")

    with tile.TileContext(nc) as tc:
        with (
            tc.tile_pool(name="persist", bufs=1) as persist,
            tc.tile_pool(name="scp", bufs=6, space="PSUM") as scp,
            tc.tile_pool(name="cpp", bufs=2, space="PSUM") as cpp,
            tc.tile_pool(name="denp", bufs=3) as denp,
            tc.tile_pool(name="wst", bufs=2) as wst,
        ):
            ident = persist.tile([128, 128], bf16, tag="ident")
            make_identity(nc, ident)
            cb_sb = persist.tile([128, NI], f32, tag="cb")
            pb_sb = persist.tile([128, NI], f32, tag="pb")
            nc.sync.dma_start(out=cb_sb, in_=cb_d[:, :])
            nc.sync.dma_start(out=pb_sb, in_=pb_d[:, :])

            kT = persist.tile([128, NI, R], bf16, tag="kT")
            v_sb = persist.tile([128, NRT, D], bf16, tag="v")
            qcb = persist.tile([128, NI, Q], bf16, tag="qcb")
            qpb = persist.tile([128, NI, Q], bf16, tag="qpb")
            rT = persist.tile([128, NI, R], bf16, tag="rT")
            outT = persist.tile([128, NI, Q], bf16, tag="outT")
            # pst ring: NSLOT separate slot tensors (separate so the
            # skewed read's conservative span stays within one slot and
            # cross-slot WAR edges don't form)
            pst = [
                persist.tile([128, 2, PIT], bf16, tag=f"pst{sl}",
                             name=f"pst{sl}")
                for sl in range(NSLOT)
            ]
            scratch = persist.tile([128, 512], bf16, tag="scratch")

            # ---- warmup: release the HAM clock gate ----
            nc.vector.memset(scratch, 0.0)
            wps = cpp.tile([128, 512], f32, tag="cp", name="wps")
            for i in range(32):
                nc.tensor.matmul(wps, lhsT=ident, rhs=scratch,
                                 start=(i == 0), stop=(i == 31))
            nc.scalar.copy(scratch, wps)

            # ---- pst init: data zeros + pad PAD_VAL (one-time) ----
            for sl in range(NSLOT):
                for hh in range(2):
                    nc.vector.memset(pst[sl][:, hh, 0:R], 0.0)
                    nc.vector.memset(pst[sl][:, hh, R:PIT], PAD_VAL)

            # ---- helpers ----
            def load_w(w_dram):
                wt = wst.tile([128, NKD, D], bf16, tag="w", name="wt")
                for kd in range(NKD):
                    nc.gpsimd.dma_start(
                        out=wt[:, kd, :],
                        in_=w_dram[kd * 128 : (kd + 1) * 128, :])
                return wt

            def wave(w_sb, i, rhs_of, emit, eng_i):
                ps = cpp.tile([128, 512], f32, tag="cp", name="ps")
                for kd in range(NKD):
                    nc.tensor.matmul(
                        ps,
                        lhsT=w_sb[:, kd, ds(i * 128, 128)],
                        rhs=rhs_of(kd),
                        start=(kd == 0),
                        stop=(kd == NKD - 1))
                emit(ps, eng_i)

            # ---- input loads (gpsimd queue order matters) ----
            with tc.tile_pool(name="inp2", bufs=1) as inp2:
                refT = inp2.tile([128, NKD, R], bf16, tag="refT")

                with tc.tile_pool(name="inp1", bufs=1) as inp1:
                    posT = inp1.tile([128, NKD, R], bf16, tag="posT")
                    qT_sb = inp1.tile([128, NKD, Q], bf16, tag="qTin")
                    for kd in range(NKD):
                        nc.gpsimd.dma_start(
                            out=posT[:, kd, :],
                            in_=posTin[kd * 128 : (kd + 1) * 128, :])
                    Wr_sb = load_w(Wr_d)
                    for kd in range(NKD):
                        nc.gpsimd.dma_start(
                            out=qT_sb[:, kd, :],
                            in_=qTin[kd * 128 : (kd + 1) * 128, :])
                    Wq_sb = load_w(Wq_d)
                    for kd in range(NKD):
                        nc.gpsimd.dma_start(
                            out=refT[:, kd, :],
                            in_=refTin[kd * 128 : (kd + 1) * 128, :])

                    # ---- rT projection ----
                    for nb in range(2):
                        for i in range(NI):
                            def em_rT(ps, e, i=i, nb=nb):
                                if e % 2:
                                    nc.vector.tensor_copy(
                                        rT[:, i, ds(nb * 512, 512)], ps)
                                else:
                                    nc.scalar.copy(
                                        rT[:, i, ds(nb * 512, 512)], ps)
                            wave(Wr_sb, i,
                                 lambda kd, nb=nb: posT[:, kd,
                                                        ds(nb * 512, 512)],
                                 em_rT, nb * NI + i)

                    # ---- q projection (+ biases) ----
                    Wv_sb = load_w(Wv_d)  # prefetch (v proj precedes kT)
                    Wk_sb = load_w(Wk_d)
                    for i in range(NI):
                        def em_q(ps, e, i=i):
                            nc.vector.tensor_scalar_add(
                                qcb[:, i, :], ps, cb_sb[:, i : i + 1])
                            nc.vector.tensor_scalar_add(
                                qpb[:, i, :], ps, pb_sb[:, i : i + 1])
                        wave(Wq_sb, i, lambda kd: qT_sb[:, kd, :], em_q, i)

                # inp1 (posT, qT) closed; C/D pools may open now
                with (
                    tc.tile_pool(name="pshp", bufs=10) as pshp,
                    tc.tile_pool(name="attnp", bufs=3) as attnp,
                    tc.tile_pool(name="attnTp", bufs=4) as attnTp,
                ):
                    # =========== attention pipeline helpers ===========
                    PP = 2 * PIT  # pst slot per-partition pitch (elems)

                    def emit_C(j, qt):
                        """position logits P for pair j, q-tile qt ->
                        pst ring slot; then skewed SBUF->SBUF reads.
                        NOTE: both copies precede both skews — a pst write
                        emitted after a skew picks up a conservative WAR
                        dep on it (the skew AP spans the whole tensor),
                        which can deadlock against psh-pool recycling."""
                        g = (j * NQT + qt) % NSLOT
                        c0 = max(0, 384 - qt * 128)  # cols < c0 never read
                        pps = {}
                        for hh in range(2):
                            off = hh * 64
                            for rb in range(2):
                                lo = c0 if rb == 0 else 0
                                pp = cpp.tile([128, 512], f32, tag="cp",
                                              name="pp")
                                nc.tensor.matmul(
                                    pp[:, lo:512],
                                    lhsT=qpb[off : off + 64, j,
                                             ds(qt * 128, 128)],
                                    rhs=rT[off : off + 64, j,
                                           rb * 512 + lo : (rb + 1) * 512],
                                    start=True, stop=True,
                                    tile_position=(off, 0))
                                pps[(hh, rb)] = pp
                        for hh in range(2):
                            for rb in range(2):
                                lo = c0 if rb == 0 else 0
                                dst = pst[g][:, hh,
                                             rb * 512 + lo : (rb + 1) * 512]
                                if hh == 0 and rb == 0 and qt % 2 == 0:
                                    nc.scalar.copy(dst,
                                                   pps[(hh, rb)][:, lo:512])
                                else:
                                    nc.vector.tensor_copy(
                                        dst, pps[(hh, rb)][:, lo:512])
                        pshs = []
                        for hh in range(2):
                            # skewed read implementing rel_shift
                            psh = pshp.tile([128, R], bf16, tag="psh",
                                            name="psh")
                            rd = bass_mod.AP(
                                tensor=pst[g].tensor,
                                offset=pst[g].offset + hh * PIT
                                + 511 - qt * 128,
                                ap=[[PP - 1, 128], [1, R]])
                            nc.gpsimd.dma_start(out=psh, in_=rd)
                            pshs.append(psh)
                        return pshs

                    attn_half = {}  # (j, hh, half) -> tile
                    den_pair = {}  # j -> den tile [128, 8]
                    aT_half = {}  # (j, hh) -> attnT tile
                    sc_pend = {}  # (j, qt) -> [sc_h0, sc_h1]

                    def emit_identity(j, qt):
                        """prime sc psums for a future step: inject shifted
                        positions via identity matmul (start of group)."""
                        pshs = psh_q.pop((j, qt))
                        scs = {}
                        for hh in range(2):
                            for rb in range(2):
                                sc = scp.tile([128, 512], f32, tag="sc",
                                              name="sc")
                                nc.tensor.matmul(
                                    sc,
                                    lhsT=ident,
                                    rhs=pshs[hh][:, ds(rb * 512, 512)],
                                    start=True, stop=False)
                                scs[(hh, rb)] = sc
                        sc_pend[(j, qt)] = scs

                    def emit_content(j, qt):
                        """content matmuls into primed scs, Exp, den."""
                        if qt == 0:
                            den_pair[j] = denp.tile([128, 8], f32, tag="den",
                                                    name="den")
                        half, sub = qt // 2, qt % 2
                        scs = sc_pend.pop((j, qt))
                        for hh in range(2):
                            off = hh * 64
                            for rb in range(2):
                                nc.tensor.matmul(
                                    scs[(hh, rb)],
                                    lhsT=qcb[off : off + 64, j,
                                             ds(qt * 128, 128)],
                                    rhs=kT[off : off + 64, j,
                                           ds(rb * 512, 512)],
                                    start=False, stop=True,
                                    tile_position=(off, 0),
                                    skip_group_check=True)
                        for hh in range(2):
                            if qt % 2 == 0:
                                attn_half[(j, hh, half)] = attnp.tile(
                                    [128, 2, R], bf16, tag="attn",
                                    name="attn")
                            at = attn_half[(j, hh, half)]
                            for rb in range(2):
                                nc.scalar.activation(
                                    at[:, sub, ds(rb * 512, 512)],
                                    scs[(hh, rb)], AF.Exp, scale=0.125)

                    def emit_den(j, qt):
                        half, sub = qt // 2, qt % 2
                        for hh in range(2):
                            at = attn_half[(j, hh, half)]
                            c = hh * 4 + qt
                            nc.vector.tensor_reduce(
                                den_pair[j][:, c : c + 1], at[:, sub, :],
                                axis=mybir.AxisListType.X,
                                op=mybir.AluOpType.add)

                    rec_pair = {}

                    def emit_norm(j, qt):
                        """recip + normalize for one q-tile; transpose a
                        finished half at qt 1/3."""
                        den = den_pair[j]
                        if qt == 0:
                            rec_pair[j] = denp.tile([128, 8], f32,
                                                    tag="den", name="rec")
                        rec = rec_pair[j]
                        half, sub = qt // 2, qt % 2
                        for hh in range(2):
                            c = hh * 4 + qt
                            nc.vector.reciprocal(rec[:, c : c + 1],
                                                 den[:, c : c + 1])
                            at = attn_half[(j, hh, half)]
                            nc.vector.tensor_scalar_mul(
                                at[:, sub, :], at[:, sub, :],
                                rec[:, c : c + 1])
                        if qt % 2 == 1:
                            for hh in range(2):
                                at = attn_half[(j, hh, half)]
                                if half == 0:
                                    aT_half[(j, hh)] = attnTp.tile(
                                        [128, NQT * NRT, 128], bf16,
                                        tag="aT", name="aT")
                                aT = aT_half[(j, hh)]
                                nc.sync.dma_start_transpose(
                                    aT[:, half * 2 * NRT :
                                       (half + 1) * 2 * NRT, :], at)

                    pv_cur = {}

                    def emit_PV_chunk(j, ck):
                        """PV quarter ck (rts 2ck, 2ck+1) for pair j;
                        pvA covers rt 0..3, pvB rt 4..7; outT = A then +B."""
                        if ck in (0, 2):
                            pv_cur[j] = scp.tile([128, 512], f32, tag="sc",
                                                 name="pv")
                        pv = pv_cur[j]
                        for rt in (2 * ck, 2 * ck + 1):
                            for hh in range(2):
                                off = hh * 64
                                h = 2 * j + hh
                                aT = aT_half[(j, hh)]
                                a4 = aT.rearrange(
                                    "p (qt rt) q -> p qt rt q", rt=NRT)
                                nc.tensor.matmul(
                                    pv[off : off + 64, :],
                                    lhsT=v_sb[:, rt, ds(h * 64, 64)],
                                    rhs=a4[:, :, rt, :],
                                    start=(rt % 4 == 0),
                                    stop=(rt % 4 == 3),
                                    tile_position=(0, off))
                        if ck == 1:
                            nc.vector.tensor_copy(outT[:, j, :], pv)
                            del pv_cur[j]
                        elif ck == 3:
                            nc.vector.tensor_tensor(
                                outT[:, j, :], outT[:, j, :], pv,
                                mybir.AluOpType.add)
                            del pv_cur[j]
                            for hh in range(2):
                                del aT_half[(j, hh)]
                                del attn_half[(j, hh, 0)]
                                del attn_half[(j, hh, 1)]

                    # =========== fused schedule ===========
                    # C(0) standalone (needs only rT/qpb slice 0)
                    psh_q = {}
                    for qt in range(NQT):
                        psh_q[(0, qt)] = emit_C(0, qt)
                    emit_identity(0, 0)

                    # kT projection i-outer; pair-0 chain + C(1) interleaved
                    def em_kT(ps, e, i, nb):
                        if e % 2:
                            nc.vector.tensor_copy(
                                kT[:, i, ds(nb * 512, 512)], ps)
                        else:
                            nc.scalar.copy(kT[:, i, ds(nb * 512, 512)], ps)

                    for i in range(NI):
                        for nb in range(2):
                            wave(Wk_sb, i,
                                 lambda kd, nb=nb: refT[:, kd,
                                                        ds(nb * 512, 512)],
                                 lambda ps, e, i=i, nb=nb: em_kT(ps, e, i,
                                                                 nb),
                                 2 * i + nb)
                        if i % 2 == 0:
                            psh_q[(1, i // 2)] = emit_C(1, i // 2)
                        else:
                            qt = (i - 1) // 2
                            emit_content(0, qt)
                            emit_den(0, qt)
                            emit_norm(0, qt)
                            if qt < NQT - 1:
                                emit_identity(0, qt + 1)
                            else:
                                emit_identity(1, 0)

                    Wo_sb = load_w(Wo_d)

                    # v projection; pair-1 chain, C(2), PV(0) interleaved
                    def em_v(ps, e, rt, nb):
                        if e % 2:
                            nc.vector.tensor_copy(
                                v_sb[:, rt, ds(nb * 512, 512)], ps)
                        else:
                            nc.scalar.copy(v_sb[:, rt, ds(nb * 512, 512)],
                                           ps)

                    vw = 0
                    for rt in range(NRT):
                        for nb in range(2):
                            ps = cpp.tile([128, 512], f32, tag="cp",
                                          name="ps")
                            for kd in range(NKD):
                                nc.tensor.matmul(
                                    ps,
                                    lhsT=refT[:, kd, ds(rt * 128, 128)],
                                    rhs=Wv_sb[:, kd, ds(nb * 512, 512)],
                                    start=(kd == 0),
                                    stop=(kd == NKD - 1))
                            em_v(ps, vw, rt, nb)
                            vw += 1
                        if rt % 2 == 1:
                            qt = rt // 2
                            emit_content(1, qt)
                            psh_q[(2, qt)] = emit_C(2, qt)
                            emit_den(1, qt)
                            emit_norm(1, qt)
                            if qt < NQT - 1:
                                emit_identity(1, qt + 1)
                            else:
                                emit_identity(2, 0)
                            emit_PV_chunk(0, qt)
                    del den_pair[0]
                    rec_pair.pop(0, None)

                    # steady pair loop
                    for j in range(2, NP):
                        for qt in range(NQT):
                            emit_content(j, qt)
                            if j + 1 < NP:
                                psh_q[(j + 1, qt)] = emit_C(j + 1, qt)
                            emit_den(j, qt)
                            emit_norm(j, qt)
                            if qt < NQT - 1:
                                emit_identity(j, qt + 1)
                            elif j + 1 < NP:
                                emit_identity(j + 1, 0)
                            emit_PV_chunk(j - 1, qt)
                        del den_pair[j - 1]
                        rec_pair.pop(j - 1, None)

                    # last pair's PV
                    for ck in range(4):
                        emit_PV_chunk(NP - 1, ck)
                    del den_pair[NP - 1]

                # ---- output projection ----
                with tc.tile_pool(name="ost", bufs=3) as ostp:
                    for qt in range(NQT):
                        for db in range(2):
                            op = scp.tile([128, 512], f32, tag="sc",
                                          name="op")
                            for i in range(NI):
                                nc.tensor.matmul(
                                    op,
                                    lhsT=outT[:, i, ds(qt * 128, 128)],
                                    rhs=Wo_sb[:, i, ds(db * 512, 512)],
                                    start=(i == 0),
                                    stop=(i == NI - 1))
                            ot = ostp.tile([128, 512], f32, tag="ot",
                                           name="ot")
                            if (qt + db) % 2:
                                nc.vector.tensor_copy(ot, op)
                            else:
                                nc.scalar.copy(ot, op)
                            nc.sync.dma_start(
                                out=out_d[qt * 128 : (qt + 1) * 128,
                                          db * 512 : (db + 1) * 512],
                                in_=ot)

    return nc


def _get_nc():
    if "nc" not in _CACHE:
        nc = _build_nc()
        if not nc.is_finalized():
            nc.finalize()
        _CACHE["nc"] = nc
    return _CACHE["nc"]


def _prep_in_maps(inputs):
    import ml_dtypes

    bf = ml_dtypes.bfloat16
    q = np.asarray(inputs["query_seqs"], dtype=np.float32)
    mem = np.asarray(inputs["memory_seqs"], dtype=np.float32)
    pos = np.asarray(inputs["positional_encoding"], dtype=np.float32)
    Wq = np.asarray(inputs["Wq"], dtype=np.float32).reshape(D, D).astype(bf)
    Wk = np.asarray(inputs["Wk"], dtype=np.float32).reshape(D, D).astype(bf)
    Wv = np.asarray(inputs["Wv"], dtype=np.float32).reshape(D, D).astype(bf)
    Wr = np.asarray(inputs["Wr"], dtype=np.float32).reshape(D, D).astype(bf)
    Wo = np.asarray(inputs["Wo"], dtype=np.float32).reshape(D, D).astype(bf)
    cb = np.ascontiguousarray(
        np.asarray(inputs["content_bias"], dtype=np.float32)
        .reshape(D).reshape(NI, 128).T)
    pb = np.ascontiguousarray(
        np.asarray(inputs["position_bias"], dtype=np.float32)
        .reshape(D).reshape(NI, 128).T)
    posT = np.ascontiguousarray(pos.T).astype(bf)

    in_maps = []
    for b in range(B):
        refT = np.ascontiguousarray(
            np.concatenate([mem[b], q[b]], axis=0).T).astype(bf)
        qT = np.ascontiguousarray(q[b].T).astype(bf)
        in_maps.append(
            dict(qT=qT, refT=refT, posT=posT,
                 Wq=Wq, Wk=Wk, Wv=Wv, Wr=Wr, Wo=Wo, cb=cb, pb=pb))
    return in_maps


def run_spmd(inputs, **kwargs):
    """Run on 8 cores; returns (output [B,Q,D], BassKernelResults)."""
    from concourse.bass_utils import run_bass_kernel_spmd

    nc = _get_nc()
    in_maps = _prep_in_maps(inputs)
    res = run_bass_kernel_spmd(nc, in_maps, core_ids=list(range(B)), **kwargs)
    out = np.stack([r["out"] for r in res.results], axis=0).astype(np.float32)
    return out, res


def kernel(**inputs) -> np.ndarray:
    out, _ = run_spmd(inputs)
    return out


# revision 32
# speedup vs baseline: 1.1358x; 1.1358x over previous
"""TransformerXL relative attention on 8 TRN2 NeuronCores (batch-parallel).

v13: single fused pipeline, rel_shift entirely on-chip, HAM-aware schedule.

Per-core (one batch element):
  - warmup matmul burst releases the HAM clock gate before real work
  - projections as 8-matmul PSUM waves: rT, q(+biases), then kT (i-outer,
    interleaved with the pair-0 attention chain), then v (interleaved with
    the pair-1 chain); weights/activations stream in bf16 via gpsimd DMA
  - position logits P per (pair, head, q-tile) -> [128,512] PSUM (never-
    read leading columns skipped) -> bf16 pst ring slots (3 separate slot
    tensors so the skewed read's conservative span stays slot-local);
    rows padded to 1536 with -30000
  - rel_shift via skewed SBUF->SBUF DMA: flat AP with partition stride
    (row_pitch - 1) reads P[q, j - q + 511]; masked region lands on the
    pad -> exp -> 0 (mask for free)
  - identity-FIRST scores: shifted P injected into [128,512] sc psums one
    step ahead (6-deep ring); content matmuls (head-pair quadrant-packed)
    close the accumulation group so each step's Exp depends only on the
    first matmuls of that step -- keeps the ACT stream dense and the PE
    out of HAM re-throttle
  - Exp on ACT (nothing else on ACT's loop lane); denominators via DVE
    tensor_reduce; batched reciprocals; normalize on DVE; attn transposed
    via DMA-xbar on the sync queue
  - PV in 4-matmul chunks spread across the next pair's steps (dedicated
    psum ring), outT drained as copy + add; output projection tail
"""

import sys

if "/opt/trn_rl_repo" not in sys.path:
    sys.path.insert(0, "/opt/trn_rl_repo")

import numpy as np

B, Q, MEM, D, H, S = 8, 512, 512, 1024, 16, 64
R = Q + MEM  # 1024
PIT = 1536  # padded row pitch of a pst slot (1024 data + 512 pad)
PAD_VAL = -30000.0
NKD = D // 128  # 8 contraction tiles
NI = D // 128  # 8 hs-tiles
NQT = Q // 128  # 4 q-tiles
NRT = R // 128  # 8 r-tiles
NP = H // 2  # 8 head pairs
NSLOT = 3  # pst ring slots

_CACHE = {}


def _build_nc():
    import concourse.bass as bass_mod
    import concourse.mybir as mybir
    import concourse.tile as tile
    from concourse import bacc
    from concourse.bass import ds
    from concourse.masks import make_identity

    f32 = mybir.dt.float32
    bf16 = mybir.dt.bfloat16
    AF = mybir.ActivationFunctionType

    nc = bacc.Bacc("TRN2", target_bir_lowering=False)

    qTin = nc.dram_tensor("qT", [D, Q], bf16, kind="ExternalInput")
    refTin = nc.dram_tensor("refT", [D, R], bf16, kind="ExternalInput")
    posTin = nc.dram_tensor("posT", [D, R], bf16, kind="ExternalInput")
    Wq_d = nc.dram_tensor("Wq", [D, D], bf16, kind="ExternalInput")
    Wk_d = nc.dram_tensor("Wk", [D, D], bf16, kind="ExternalInput")
    Wv_d = nc.dram_tensor("Wv", [D, D], bf16, kind="ExternalInput")
    Wr_d = nc.dram_tensor("Wr", [D, D], bf16, kind="ExternalInput")
    Wo_d = nc.dram_tensor("Wo", [D, D], bf16, kind="ExternalInput")
    cb_d = nc.dram_tensor("cb", [128, NI], f32, kind="ExternalInput")
    pb_d = nc.dram_tensor("pb", [128, NI], f32, kind="ExternalInput")
    out_d = nc.dram_tensor("out", [Q, D], f32, kind="ExternalOutput")

    with tile.TileContext(nc) as tc:
        with (
            tc.tile_pool(name="persist", bufs=1) as persist,
            tc.tile_pool(name="scp", bufs=6, space="PSUM") as scp,
            tc.tile_pool(name="cpp", bufs=2, space="PSUM") as cpp,
            tc.tile_pool(name="denp", bufs=3) as denp,
            tc.tile_pool(name="wst", bufs=2) as wst,
        ):
            ident = persist.tile([128, 128], bf16, tag="ident")
            make_identity(nc, ident)
            cb_sb = persist.tile([128, NI], f32, tag="cb")
            pb_sb = persist.tile([128, NI], f32, tag="pb")
            nc.sync.dma_start(out=cb_sb, in_=cb_d[:, :])
            nc.sync.dma_start(out=pb_sb, in_=pb_d[:, :])

            kT = persist.tile([128, NI, R], bf16, tag="kT")
            v_sb = persist.tile([128, NRT, D], bf16, tag="v")
            qcb = persist.tile([128, NI, Q], bf16, tag="qcb")
            qpb = persist.tile([128, NI, Q], bf16, tag="qpb")
            rT = persist.tile([128, NI, R], bf16, tag="rT")
            outT = persist.tile([128, NI, Q], bf16, tag="outT")
            # pst ring: NSLOT separate slot tensors (separate so the
            # skewed read's conservative span stays within one slot and
            # cross-slot WAR edges don't form)
            pst = [
                persist.tile([128, 2, PIT], bf16, tag=f"pst{sl}",
                             name=f"pst{sl}")
                for sl in range(NSLOT)
            ]
            scratch = persist.tile([128, 512], bf16, tag="scratch")

            # ---- warmup: release the HAM clock gate ----
            nc.vector.memset(scratch, 0.0)
            wps = cpp.tile([128, 512], f32, tag="cp", name="wps")
            for i in range(32):
                nc.tensor.matmul(wps, lhsT=ident, rhs=scratch,
                                 start=(i == 0), stop=(i == 31))
            nc.scalar.copy(scratch, wps)

            # ---- pst init: data zeros + pad PAD_VAL (one-time) ----
            for sl in range(NSLOT):
                for hh in range(2):
                    nc.vector.memset(pst[sl][:, hh, 0:R], 0.0)
                    nc.vector.memset(pst[sl][:, hh, R:PIT], PAD_VAL)

            # ---- helpers ----
            def load_w(w_dram):
                wt = wst.tile([128, NKD, D], bf16, tag="w", name="wt")
                for kd in range(NKD):
                    nc.gpsimd.dma_start(
                        out=wt[:, kd, :],
                        in_=w_dram[kd * 128 : (kd + 1) * 128, :])
                return wt

            def wave(w_sb, i, rhs_of, emit, eng_i):
                ps = cpp.tile([128, 512], f32, tag="cp", name="ps")
                for kd in range(NKD):
                    nc.tensor.matmul(
                        ps,
                        lhsT=w_sb[:, kd, ds(i * 128, 128)],
                        rhs=rhs_of(kd),
                        start=(kd == 0),
                        stop=(kd == NKD - 1))
                emit(ps, eng_i)

            # ---- input loads (gpsimd queue order matters) ----
            with tc.tile_pool(name="inp2", bufs=1) as inp2:
                refT = inp2.tile([128, NKD, R], bf16, tag="refT")

                with tc.tile_pool(name="inp1", bufs=1) as inp1:
                    posT = inp1.tile([128, NKD, R], bf16, tag="posT")
                    qT_sb = inp1.tile([128, NKD, Q], bf16, tag="qTin")
                    for kd in range(NKD):
                        nc.gpsimd.dma_start(
                            out=posT[:, kd, :],
                            in_=posTin[kd * 128 : (kd + 1) * 128, :])
                    Wr_sb = load_w(Wr_d)
                    for kd in range(NKD):
                        nc.gpsimd.dma_start(
                            out=qT_sb[:, kd, :],
                            in_=qTin[kd * 128 : (kd + 1) * 128, :])
                    Wq_sb = load_w(Wq_d)
                    for kd in range(NKD):
                        nc.gpsimd.dma_start(
                            out=refT[:, kd, :],
                            in_=refTin[kd * 128 : (kd + 1) * 128, :])

                    # ---- rT projection ----
                    for nb in range(2):
                        for i in range(NI):
                            def em_rT(ps, e, i=i, nb=nb):
                                if e % 2:
                                    nc.vector.tensor_copy(
                                        rT[:, i, ds(nb * 512, 512)], ps)
                                else:
                                    nc.scalar.copy(
                                        rT[:, i, ds(nb * 512, 512)], ps)
                            wave(Wr_sb, i,
                                 lambda kd, nb=nb: posT[:, kd,
                                                        ds(nb * 512, 512)],
                                 em_rT, nb * NI + i)

                    # ---- q projection (+ biases) ----
                    Wv_sb = load_w(Wv_d)  # prefetch (v proj precedes kT)
                    Wk_sb = load_w(Wk_d)
                    for i in range(NI):
                        def em_q(ps, e, i=i):
                            nc.vector.tensor_scalar_add(
                                qcb[:, i, :], ps, cb_sb[:, i : i + 1])
                            nc.vector.tensor_scalar_add(
                                qpb[:, i, :], ps, pb_sb[:, i : i + 1])
                        wave(Wq_sb, i, lambda kd: qT_sb[:, kd, :], em_q, i)

                # inp1 (posT, qT) closed; C/D pools may open now
                with (
                    tc.tile_pool(name="pshp", bufs=10) as pshp,
                    tc.tile_pool(name="attnp", bufs=3) as attnp,
                    tc.tile_pool(name="attnTp", bufs=4) as attnTp,
                ):
                    # =========== attention pipeline helpers ===========
                    PP = 2 * PIT  # pst slot per-partition pitch (elems)

                    def emit_C(j, qt):
                        """position logits P for pair j, q-tile qt ->
                        pst ring slot; then skewed SBUF->SBUF reads.
                        NOTE: both copies precede both skews — a pst write
                        emitted after a skew picks up a conservative WAR
                        dep on it (the skew AP spans the whole tensor),
                        which can deadlock against psh-pool recycling."""
                        g = (j * NQT + qt) % NSLOT
                        c0 = max(0, 384 - qt * 128)  # cols < c0 never read
                        pps = {}
                        for hh in range(2):
                            off = hh * 64
                            for rb in range(2):
                                lo = c0 if rb == 0 else 0
                                pp = cpp.tile([128, 512], f32, tag="cp",
                                              name="pp")
                                nc.tensor.matmul(
                                    pp[:, lo:512],
                                    lhsT=qpb[off : off + 64, j,
                                             ds(qt * 128, 128)],
                                    rhs=rT[off : off + 64, j,
                                           rb * 512 + lo : (rb + 1) * 512],
                                    start=True, stop=True,
                                    tile_position=(off, 0))
                                pps[(hh, rb)] = pp
                        for hh in range(2):
                            for rb in range(2):
                                lo = c0 if rb == 0 else 0
                                dst = pst[g][:, hh,
                                             rb * 512 + lo : (rb + 1) * 512]
                                if hh == 0 and rb == 0 and qt % 2 == 0:
                                    nc.scalar.copy(dst,
                                                   pps[(hh, rb)][:, lo:512])
                                else:
                                    nc.vector.tensor_copy(
                                        dst, pps[(hh, rb)][:, lo:512])
                        pshs = []
                        for hh in range(2):
                            # skewed read implementing rel_shift
                            psh = pshp.tile([128, R], bf16, tag="psh",
                                            name="psh")
                            rd = bass_mod.AP(
                                tensor=pst[g].tensor,
                                offset=pst[g].offset + hh * PIT
                                + 511 - qt * 128,
                                ap=[[PP - 1, 128], [1, R]])
                            nc.gpsimd.dma_start(out=psh, in_=rd)
                            pshs.append(psh)
                        return pshs

                    attn_half = {}  # (j, hh, half) -> tile
                    den_pair = {}  # j -> den tile [128, 8]
                    aT_half = {}  # (j, hh) -> attnT tile
                    sc_pend = {}  # (j, qt) -> [sc_h0, sc_h1]

                    def emit_identity(j, qt):
                        """prime sc psums for a future step: inject shifted
                        positions via identity matmul (start of group)."""
                        pshs = psh_q.pop((j, qt))
                        scs = {}
                        for hh in range(2):
                            for rb in range(2):
                                sc = scp.tile([128, 512], f32, tag="sc",
                                              name="sc")
                                nc.tensor.matmul(
                                    sc,
                                    lhsT=ident,
                                    rhs=pshs[hh][:, ds(rb * 512, 512)],
                                    start=True, stop=False)
                                scs[(hh, rb)] = sc
                        sc_pend[(j, qt)] = scs

                    def emit_content(j, qt):
                        """content matmuls into primed scs, Exp, den."""
                        if qt == 0:
                            den_pair[j] = denp.tile([128, 8], f32, tag="den",
                                                    name="den")
                        half, sub = qt // 2, qt % 2
                        scs = sc_pend.pop((j, qt))
                        for hh in range(2):
                            off = hh * 64
                            for rb in range(2):
                                nc.tensor.matmul(
                                    scs[(hh, rb)],
                                    lhsT=qcb[off : off + 64, j,
                                             ds(qt * 128, 128)],
                                    rhs=kT[off : off + 64, j,
                                           ds(rb * 512, 512)],
                                    start=False, stop=True,
                                    tile_position=(off, 0),
                                    skip_group_check=True)
                        for hh in range(2):
                            if qt % 2 == 0:
                                attn_half[(j, hh, half)] = attnp.tile(
                                    [128, 2, R], bf16, tag="attn",
                                    name="attn")
                            at = attn_half[(j, hh, half)]
                            for rb in range(2):
                                nc.scalar.activation(
                                    at[:, sub, ds(rb * 512, 512)],
                                    scs[(hh, rb)], AF.Exp, scale=0.125)

                    def emit_den(j, qt):
                        half, sub = qt // 2, qt % 2
                        for hh in range(2):
                            at = attn_half[(j, hh, half)]
                            c = hh * 4 + qt
                            nc.vector.tensor_reduce(
                                den_pair[j][:, c : c + 1], at[:, sub, :],
                                axis=mybir.AxisListType.X,
                                op=mybir.AluOpType.add)

                    rec_pair = {}

                    def emit_norm(j, qt):
                        """recip + normalize for one q-tile; transpose a
                        finished half at qt 1/3."""
                        den = den_pair[j]
                        if qt == 0:
                            rec_pair[j] = denp.tile([128, 8], f32,
                                                    tag="den", name="rec")
                        rec = rec_pair[j]
                        half, sub = qt // 2, qt % 2
                        for hh in range(2):
                            c = hh * 4 + qt
                            nc.vector.reciprocal(rec[:, c : c + 1],
                                                 den[:, c : c + 1])
                            at = attn_half[(j, hh, half)]
                            nc.vector.tensor_scalar_mul(
                                at[:, sub, :], at[:, sub, :],
                                rec[:, c : c + 1])
                        if qt % 2 == 1:
                            for hh in range(2):
                                at = attn_half[(j, hh, half)]
                                if half == 0:
                                    aT_half[(j, hh)] = attnTp.tile(
                                        [128, NQT * NRT, 128], bf16,
                                        tag="aT", name="aT")
                                aT = aT_half[(j, hh)]
                                nc.sync.dma_start_transpose(
                                    aT[:, half * 2 * NRT :
                                       (half + 1) * 2 * NRT, :], at)

                    pv_cur = {}

                    def emit_PV_chunk(j, ck):
                        """PV quarter ck (rts 2ck, 2ck+1) for pair j;
                        pvA covers rt 0..3, pvB rt 4..7; outT = A then +B."""
                        if ck in (0, 2):
                            pv_cur[j] = scp.tile([128, 512], f32, tag="sc",
                                                 name="pv")
                        pv = pv_cur[j]
                        for rt in (2 * ck, 2 * ck + 1):
                            for hh in range(2):
                                off = hh * 64
                                h = 2 * j + hh
                                aT = aT_half[(j, hh)]
                                a4 = aT.rearrange(
                                    "p (qt rt) q -> p qt rt q", rt=NRT)
                                nc.tensor.matmul(
                                    pv[off : off + 64, :],
                                    lhsT=v_sb[:, rt, ds(h * 64, 64)],
                                    rhs=a4[:, :, rt, :],
                                    start=(rt % 4 == 0),
                                    stop=(rt % 4 == 3),
                                    tile_position=(0, off))
                        if ck == 1:
                            nc.vector.tensor_copy(outT[:, j, :], pv)
                            del pv_cur[j]
                        elif ck == 3:
                            nc.vector.tensor_tensor(
                                outT[:, j, :], outT[:, j, :], pv,
                                mybir.AluOpType.add)
                            del pv_cur[j]
                            for hh in range(2):
                                del aT_half[(j, hh)]
                                del attn_half[(j, hh, 0)]
                                del attn_half[(j, hh, 1)]

                    # =========== fused schedule ===========
                    # C(0) standalone (needs only rT/qpb slice 0)
                    psh_q = {}
                    for qt in range(NQT):
                        psh_q[(0, qt)] = emit_C(0, qt)
                    emit_identity(0, 0)

                    # kT projection i-outer; pair-0 chain + C(1) interleaved
                    def em_kT(ps, e, i, nb):
                        if e % 2:
                            nc.vector.tensor_copy(
                                kT[:, i, ds(nb * 512, 512)], ps)
                        else:
                            nc.scalar.copy(kT[:, i, ds(nb * 512, 512)], ps)

                    for i in range(NI):
                        for nb in range(2):
                            wave(Wk_sb, i,
                                 lambda kd, nb=nb: refT[:, kd,
                                                        ds(nb * 512, 512)],
                                 lambda ps, e, i=i, nb=nb: em_kT(ps, e, i,
                                                                 nb),
                                 2 * i + nb)
                        if i % 2 == 0:
                            psh_q[(1, i // 2)] = emit_C(1, i // 2)
                        else:
                            qt = (i - 1) // 2
                            emit_content(0, qt)
                            emit_den(0, qt)
                            emit_norm(0, qt)
                            if qt < NQT - 1:
                                emit_identity(0, qt + 1)
                            else:
                                emit_identity(1, 0)

                    Wo_sb = load_w(Wo_d)

                    # v projection; pair-1 chain, C(2), PV(0) interleaved
                    def em_v(ps, e, rt, nb):
                        if e % 2:
                            nc.vector.tensor_copy(
                                v_sb[:, rt, ds(nb * 512, 512)], ps)
                        else:
                            nc.scalar.copy(v_sb[:, rt, ds(nb * 512, 512)],
                                           ps)

                    vw = 0
                    for rt in range(NRT):
                        for nb in range(2):
                            ps = cpp.tile([128, 512], f32, tag="cp",
                                          name="ps")
                            for kd in range(NKD):
                                nc.tensor.matmul(
                                    ps,
                                    lhsT=refT[:, kd, ds(rt * 128, 128)],
                                    rhs=Wv_sb[:, kd, ds(nb * 512, 512)],
                                    start=(kd == 0),
                                    stop=(kd == NKD - 1))
                            em_v(ps, vw, rt, nb)
                            vw += 1
                        if rt % 2 == 1:
                            qt = rt // 2
                            emit_content(1, qt)
                            psh_q[(2, qt)] = emit_C(2, qt)
                            emit_den(1, qt)
                            emit_norm(1, qt)
                            if qt < NQT - 1:
                                emit_identity(1, qt + 1)
                            else:
                                emit_identity(2, 0)
                            emit_PV_chunk(0, qt)
                    del den_pair[0]
                    rec_pair.pop(0, None)

                    # steady pair loop
                    for j in range(2, NP):
                        for qt in range(NQT):
                            emit_content(j, qt)
                            if j + 1 < NP:
                                psh_q[(j + 1, qt)] = emit_C(j + 1, qt)
                            emit_den(j, qt)
                            emit_norm(j, qt)
                            if qt < NQT - 1:
                                emit_identity(j, qt + 1)
                            elif j + 1 < NP:
                                emit_identity(j + 1, 0)
                            emit_PV_chunk(j - 1, qt)
                        del den_pair[j - 1]
                        rec_pair.pop(j - 1, None)

                    # last pair's PV
                    for ck in range(4):
                        emit_PV_chunk(NP - 1, ck)
                    del den_pair[NP - 1]

                # ---- output projection ----
                with tc.tile_pool(name="ost", bufs=3) as ostp:
                    for qt in range(NQT):
                        for db in range(2):
                            op = cpp.tile([128, 512], f32, tag="cp",
                                          name="op")
                            for i in range(NI):
                                nc.tensor.matmul(
                                    op,
                                    lhsT=outT[:, i, ds(qt * 128, 128)],
                                    rhs=Wo_sb[:, i, ds(db * 512, 512)],
                                    start=(i == 0),
                                    stop=(i == NI - 1))
                            ot = ostp.tile([128, 512], f32, tag="ot",
                                           name="ot")
                            if (qt + db) % 2:
                                nc.vector.tensor_copy(ot, op)
                            else:
                                nc.scalar.copy(ot, op)
                            nc.sync.dma_start(
                                out=out_d[qt * 128 : (qt + 1) * 128,
                                          db * 512 : (db + 1) * 512],
                                in_=ot)

    return nc


def _get_nc():
    if "nc" not in _CACHE:
        nc = _build_nc()
        if not nc.is_finalized():
            nc.finalize()
        _CACHE["nc"] = nc
    return _CACHE["nc"]


def _prep_in_maps(inputs):
    import ml_dtypes

    bf = ml_dtypes.bfloat16
    q = np.asarray(inputs["query_seqs"], dtype=np.float32)
    mem = np.asarray(inputs["memory_seqs"], dtype=np.float32)
    pos = np.asarray(inputs["positional_encoding"], dtype=np.float32)
    Wq = np.asarray(inputs["Wq"], dtype=np.float32).reshape(D, D).astype(bf)
    Wk = np.asarray(inputs["Wk"], dtype=np.float32).reshape(D, D).astype(bf)
    Wv = np.asarray(inputs["Wv"], dtype=np.float32).reshape(D, D).astype(bf)
    Wr = np.asarray(inputs["Wr"], dtype=np.float32).reshape(D, D).astype(bf)
    Wo = np.asarray(inputs["Wo"], dtype=np.float32).reshape(D, D).astype(bf)
    cb = np.ascontiguousarray(
        np.asarray(inputs["content_bias"], dtype=np.float32)
        .reshape(D).reshape(NI, 128).T)
    pb = np.ascontiguousarray(
        np.asarray(inputs["position_bias"], dtype=np.float32)
        .reshape(D).reshape(NI, 128).T)
    posT = np.ascontiguousarray(pos.T).astype(bf)

    in_maps = []
    for b in range(B):
        refT = np.ascontiguousarray(
            np.concatenate([mem[b], q[b]], axis=0).T).astype(bf)
        qT = np.ascontiguousarray(q[b].T).astype(bf)
        in_maps.append(
            dict(qT=qT, refT=refT, posT=posT,
                 Wq=Wq, Wk=Wk, Wv=Wv, Wr=Wr, Wo=Wo, cb=cb, pb=pb))
    return in_maps


def run_spmd(inputs, **kwargs):
    """Run on 8 cores; returns (output [B,Q,D], BassKernelResults)."""
    from concourse.bass_utils import run_bass_kernel_spmd

    nc = _get_nc()
    in_maps = _prep_in_maps(inputs)
    res = run_bass_kernel_spmd(nc, in_maps, core_ids=list(range(B)), **kwargs)
    out = np.stack([r["out"] for r in res.results], axis=0).astype(np.float32)
    return out, res


def kernel(**inputs) -> np.ndarray:
    out, _ = run_spmd(inputs)
    return out
